# revision 7
# baseline (speedup 1.0000x reference)
"""Trainium2 Bass kernel for Block (2D overlapping patch extraction).

x: [4, 8, 512, 512] f32 -> out: [4, 8, 3969, 16, 16]
block 16x16, stride 8x8, 'valid' -> nbh = nbw = 63.

Sharding: data-parallel over the 32 (batch*channel) images; 4 images per
NeuronCore across 8 cores. No cross-core communication.

Per-core plan (images c in [0,4), block rows i in [0,63)):
 - i is processed in two chunks (32 + 31) so that (c, i) fits in <=128 SBUF
   partitions.
 - Load: partition (c*nI + ii) holds input rows [8*(i0+ii), 8*(i0+ii)+16) of
   image c -- a 32 KiB contiguous DRAM run per partition (rows are read ~2x
   due to the stride-8 overlap; this buys fully-contiguous DMA descriptors).
 - Rearrange on-chip: 4 vector-engine copies (one per (a, b) quadrant of the
   2x2 cell decomposition of a 16x16 block) gather the patch layout
   out[j, a*8+h, b*8+w] = in[8a+h, 8(j+b)+w] within each partition.
 - Store: partition (c*nI + ii) now holds out[c, i*63:(i+1)*63, :, :]
   verbatim -- a 63 KiB contiguous DRAM run per partition.
"""

import numpy as np

NCORES = 8
CH_PER_CORE = 4
H = W = 512
BH = BW = 16
SH = SW = 8
NB = 63          # blocks per axis
ROW = W          # elements per image row
IMG = H * W      # elements per image
OUT_BLK = BH * BW            # 256 elements per block
OUT_ROWCH = NB * OUT_BLK     # 16128 elements per block-row
OUT_IMG = NB * NB * OUT_BLK  # elements per output image

_CACHE = {}


def _build_nc():
    import concourse.bass as bass
    import concourse.bacc as bacc
    import concourse.mybir as mybir
    from concourse import tile

    nc = bacc.Bacc(
        "TRN2", target_bir_lowering=False, debug=False, num_devices=NCORES
    )
    xs = nc.dram_tensor(
        "xs", [CH_PER_CORE, H, W], mybir.dt.float32, kind="ExternalInput"
    )
    out = nc.dram_tensor(
        "out",
        [CH_PER_CORE, NB * NB, BH, BW],
        mybir.dt.float32,
        kind="ExternalOutput",
    )
    out_rows = out.rearrange("c (i j) h w -> c i (j h w)", i=NB)

    GRP = SH * ROW  # 4096 elements: one 8-row group
    with tile.TileContext(nc) as tc:
        with (
            tc.tile_pool(name="lp", bufs=2) as lp,
            tc.tile_pool(name="op", bufs=2) as op,
        ):
            for i0, nI in ((0, 32), (32, 31)):
                P = CH_PER_CORE * nI
                L = lp.tile([128, 2 * GRP], mybir.dt.float32, name=f"L{i0}", tag="L")
                # Partition p = ii*4 + c (row-group-major): the DRAM AP's
                # outermost dim count drives the SDMA engine spray -- outer
                # count 4 pins a DMA to 4 engines (~67GB/s), outer count >=16
                # engages all 16 (~400GB/s).  Two 16KB-per-partition loads
                # (halves a=0 / a=1 of the 16-row window; >16KB descriptors
                # also run at half the per-engine rate).
                for a in (0, 1):
                    src = bass.AP(
                        xs,
                        (i0 + a) * GRP,
                        [[GRP, nI], [IMG, CH_PER_CORE], [1, GRP]],
                    )
                    nc.gpsimd.dma_start(out=L[:P, a * GRP : (a + 1) * GRP], in_=src)

                O = op.tile([128, OUT_ROWCH], mybir.dt.float32, name=f"O{i0}", tag="O")
                O_r = O[:P, :].rearrange(
                    "p (j A h B w) -> p A B j h w", j=NB, A=2, h=SH, B=2, w=SW
                )
                L_r = L[:P, :].rearrange("p (A h col) -> p A h col", A=2, h=SH, col=ROW)
                for a in (0, 1):
                    for b in (0, 1):
                        dst_ap = O_r[:, a, b]
                        src_ap = L_r[:, a, :, SW * b : SW * b + NB * SW].rearrange(
                            "p h (j w) -> p j h w", w=SW
                        )
                        nc.vector.tensor_copy(out=dst_ap, in_=src_ap)

                # Four ~16KB-per-partition stores (j-groups), spread across the
                # two HWDGE queues + SWDGE so all rings stay fed.
                engines = [nc.sync, nc.scalar, nc.gpsimd, nc.sync]
                for gi, (j0, njg) in enumerate(((0, 16), (16, 16), (32, 16), (48, 15))):
                    eng = engines[gi]
                    eng.dma_start(
                        out=bass.AP(
                            out,
                            i0 * OUT_ROWCH + j0 * OUT_BLK,
                            [
                                [OUT_ROWCH, nI],
                                [OUT_IMG, CH_PER_CORE],
                                [1, njg * OUT_BLK],
                            ],
                        ),
                        in_=O[:P, j0 * OUT_BLK : (j0 + njg) * OUT_BLK],
                    )
    nc.compile()
    return nc


def get_nc():
    if "nc" not in _CACHE:
        _CACHE["nc"] = _build_nc()
    return _CACHE["nc"]


def _enable_jax_compile_cache():
    try:
        import jax

        jax.config.update("jax_compilation_cache_dir", "/tmp/jax_neff_cache")
        jax.config.update("jax_persistent_cache_min_entry_size_bytes", -1)
        jax.config.update("jax_persistent_cache_min_compile_time_secs", 0.0)
    except Exception:
        pass


def run_spmd(in_maps, **kwargs):
    from concourse.bass_utils import run_bass_kernel_spmd

    _enable_jax_compile_cache()
    return run_bass_kernel_spmd(
        get_nc(), in_maps, core_ids=list(range(NCORES)), **kwargs
    )


def make_in_maps(x: np.ndarray):
    xs = np.asarray(x, dtype=np.float32).reshape(-1, H, W)
    return [
        {"xs": np.ascontiguousarray(xs[c * CH_PER_CORE : (c + 1) * CH_PER_CORE])}
        for c in range(NCORES)
    ]


def assemble(results, batch_shape):
    outs = np.stack([r["out"] for r in results])  # [8, 4, 3969, 16, 16]
    return outs.reshape(*batch_shape, NB * NB, BH, BW)


def kernel(**inputs) -> np.ndarray:
    x = np.asarray(inputs["x"])
    res = run_spmd(make_in_maps(x))
    return assemble(res.results, x.shape[:2])


# revision 8
# speedup vs baseline: 4.3409x; 4.3409x over previous
"""Trainium2 Bass kernel for Block (2D overlapping patch extraction).

x: [4, 8, 512, 512] f32 -> out: [4, 8, 3969, 16, 16]
block 16x16, stride 8x8, 'valid' -> nbh = nbw = 63.

Sharding: data-parallel over the 32 (batch*channel) images; 4 images per
NeuronCore across 8 cores. No cross-core communication.

Per-core plan (images c in [0,4), block rows i in [0,63)):
 - i is processed in two chunks (32 + 31) so that (c, i) fits in <=128 SBUF
   partitions.
 - Load: partition (c*nI + ii) holds input rows [8*(i0+ii), 8*(i0+ii)+16) of
   image c -- a 32 KiB contiguous DRAM run per partition (rows are read ~2x
   due to the stride-8 overlap; this buys fully-contiguous DMA descriptors).
 - Rearrange on-chip: 4 vector-engine copies (one per (a, b) quadrant of the
   2x2 cell decomposition of a 16x16 block) gather the patch layout
   out[j, a*8+h, b*8+w] = in[8a+h, 8(j+b)+w] within each partition.
 - Store: partition (c*nI + ii) now holds out[c, i*63:(i+1)*63, :, :]
   verbatim -- a 63 KiB contiguous DRAM run per partition.
"""

import numpy as np

NCORES = 8
CH_PER_CORE = 4
H = W = 512
BH = BW = 16
SH = SW = 8
NB = 63          # blocks per axis
ROW = W          # elements per image row
IMG = H * W      # elements per image
OUT_BLK = BH * BW            # 256 elements per block
OUT_ROWCH = NB * OUT_BLK     # 16128 elements per block-row
OUT_IMG = NB * NB * OUT_BLK  # elements per output image

_CACHE = {}


def _build_nc():
    import concourse.bass as bass
    import concourse.bacc as bacc
    import concourse.mybir as mybir
    from concourse import tile

    nc = bacc.Bacc(
        "TRN2", target_bir_lowering=False, debug=False, num_devices=NCORES
    )
    xs = nc.dram_tensor(
        "xs", [CH_PER_CORE, H, W], mybir.dt.float32, kind="ExternalInput"
    )
    out = nc.dram_tensor(
        "out",
        [CH_PER_CORE, NB * NB, BH, BW],
        mybir.dt.float32,
        kind="ExternalOutput",
    )
    out_rows = out.rearrange("c (i j) h w -> c i (j h w)", i=NB)

    GRP = SH * ROW  # 4096 elements: one 8-row group
    with tile.TileContext(nc) as tc:
        with (
            tc.tile_pool(name="lp", bufs=2) as lp,
            tc.tile_pool(name="op", bufs=2) as op,
        ):
            # Both chunks are 32 block-rows (overlapping at i=31, which is
            # stored twice with identical bytes): the DMA engine spray only
            # engages all 16 engines when the AP outer count is a multiple
            # of 16 -- a 31-row chunk ran on 1-2 engines.
            for i0, nI in ((0, 32), (31, 32)):
                P = CH_PER_CORE * nI
                L = lp.tile([128, 2 * GRP], mybir.dt.float32, name=f"L{i0}", tag="L")
                # Partition p = ii*4 + c (row-group-major): the DRAM AP's
                # outermost dim count drives the SDMA engine spray -- outer
                # count 4 pins a DMA to 4 engines (~67GB/s), outer count >=16
                # engages all 16 (~400GB/s).  Two 16KB-per-partition loads
                # (halves a=0 / a=1 of the 16-row window; >16KB descriptors
                # also run at half the per-engine rate).
                for a in (0, 1):
                    src = bass.AP(
                        xs,
                        (i0 + a) * GRP,
                        [[GRP, nI], [IMG, CH_PER_CORE], [1, GRP]],
                    )
                    nc.gpsimd.dma_start(out=L[:P, a * GRP : (a + 1) * GRP], in_=src)

                O = op.tile([128, OUT_ROWCH], mybir.dt.float32, name=f"O{i0}", tag="O")
                O_r = O[:P, :].rearrange(
                    "p (j A h B w) -> p A B j h w", j=NB, A=2, h=SH, B=2, w=SW
                )
                L_r = L[:P, :].rearrange("p (A h col) -> p A h col", A=2, h=SH, col=ROW)
                for a in (0, 1):
                    for b in (0, 1):
                        dst_ap = O_r[:, a, b]
                        src_ap = L_r[:, a, :, SW * b : SW * b + NB * SW].rearrange(
                            "p h (j w) -> p j h w", w=SW
                        )
                        nc.vector.tensor_copy(out=dst_ap, in_=src_ap)

                # Four ~16KB-per-partition stores (j-groups), spread across the
                # two HWDGE queues + SWDGE so all rings stay fed.
                engines = [nc.sync, nc.scalar, nc.gpsimd, nc.sync]
                for gi, (j0, njg) in enumerate(((0, 16), (16, 16), (32, 16), (48, 15))):
                    eng = engines[gi]
                    eng.dma_start(
                        out=bass.AP(
                            out,
                            i0 * OUT_ROWCH + j0 * OUT_BLK,
                            [
                                [OUT_ROWCH, nI],
                                [OUT_IMG, CH_PER_CORE],
                                [1, njg * OUT_BLK],
                            ],
                        ),
                        in_=O[:P, j0 * OUT_BLK : (j0 + njg) * OUT_BLK],
                    )
    nc.compile()
    return nc


def get_nc():
    if "nc" not in _CACHE:
        _CACHE["nc"] = _build_nc()
    return _CACHE["nc"]


def _enable_jax_compile_cache():
    try:
        import jax

        jax.config.update("jax_compilation_cache_dir", "/tmp/jax_neff_cache")
        jax.config.update("jax_persistent_cache_min_entry_size_bytes", -1)
        jax.config.update("jax_persistent_cache_min_compile_time_secs", 0.0)
    except Exception:
        pass


def run_spmd(in_maps, **kwargs):
    from concourse.bass_utils import run_bass_kernel_spmd

    _enable_jax_compile_cache()
    return run_bass_kernel_spmd(
        get_nc(), in_maps, core_ids=list(range(NCORES)), **kwargs
    )


def make_in_maps(x: np.ndarray):
    xs = np.asarray(x, dtype=np.float32).reshape(-1, H, W)
    return [
        {"xs": np.ascontiguousarray(xs[c * CH_PER_CORE : (c + 1) * CH_PER_CORE])}
        for c in range(NCORES)
    ]


def assemble(results, batch_shape):
    outs = np.stack([r["out"] for r in results])  # [8, 4, 3969, 16, 16]
    return outs.reshape(*batch_shape, NB * NB, BH, BW)


def kernel(**inputs) -> np.ndarray:
    x = np.asarray(inputs["x"])
    res = run_spmd(make_in_maps(x))
    return assemble(res.results, x.shape[:2])


# revision 9
# speedup vs baseline: 5.4390x; 1.2530x over previous
"""Trainium2 Bass kernel for Block (2D overlapping patch extraction).

x: [4, 8, 512, 512] f32 -> out: [4, 8, 3969, 16, 16]
block 16x16, stride 8x8, 'valid' -> nbh = nbw = 63.

Sharding: data-parallel over the 32 (batch*channel) images; 4 images per
NeuronCore across 8 cores. No cross-core communication.

Per-core plan (images c in [0,4), block rows i in [0,63)):
 - i is processed in two chunks (32 + 31) so that (c, i) fits in <=128 SBUF
   partitions.
 - Load: partition (c*nI + ii) holds input rows [8*(i0+ii), 8*(i0+ii)+16) of
   image c -- a 32 KiB contiguous DRAM run per partition (rows are read ~2x
   due to the stride-8 overlap; this buys fully-contiguous DMA descriptors).
 - Rearrange on-chip: 4 vector-engine copies (one per (a, b) quadrant of the
   2x2 cell decomposition of a 16x16 block) gather the patch layout
   out[j, a*8+h, b*8+w] = in[8a+h, 8(j+b)+w] within each partition.
 - Store: partition (c*nI + ii) now holds out[c, i*63:(i+1)*63, :, :]
   verbatim -- a 63 KiB contiguous DRAM run per partition.
"""

import numpy as np

NCORES = 8
CH_PER_CORE = 4
H = W = 512
BH = BW = 16
SH = SW = 8
NB = 63          # blocks per axis
ROW = W          # elements per image row
IMG = H * W      # elements per image
OUT_BLK = BH * BW            # 256 elements per block
OUT_ROWCH = NB * OUT_BLK     # 16128 elements per block-row
OUT_IMG = NB * NB * OUT_BLK  # elements per output image

_CACHE = {}


def _build_nc():
    import concourse.bass as bass
    import concourse.bacc as bacc
    import concourse.mybir as mybir
    from concourse import tile

    nc = bacc.Bacc(
        "TRN2", target_bir_lowering=False, debug=False, num_devices=NCORES
    )
    xs = nc.dram_tensor(
        "xs", [CH_PER_CORE, H, W], mybir.dt.float32, kind="ExternalInput"
    )
    out = nc.dram_tensor(
        "out",
        [CH_PER_CORE, NB * NB, BH, BW],
        mybir.dt.float32,
        kind="ExternalOutput",
    )
    out_rows = out.rearrange("c (i j) h w -> c i (j h w)", i=NB)

    GRP = SH * ROW  # 4096 elements: one 8-row group
    with tile.TileContext(nc) as tc:
        with (
            tc.tile_pool(name="lp", bufs=2) as lp,
            tc.tile_pool(name="op", bufs=2) as op,
        ):
            # Both chunks are 32 block-rows (overlapping at i=31, which is
            # stored twice with identical bytes): the DMA engine spray only
            # engages all 16 engines when the AP outer count is a multiple
            # of 16 -- a 31-row chunk ran on 1-2 engines.
            for i0, nI in ((0, 32), (31, 32)):
                P = CH_PER_CORE * nI
                L = lp.tile([128, 2 * GRP], mybir.dt.float32, name=f"L{i0}", tag="L")
                # Partition p = ii*4 + c (row-group-major): the DRAM AP's
                # outermost dim count drives the SDMA engine spray -- outer
                # count 4 pins a DMA to 4 engines (~67GB/s), outer count >=16
                # engages all 16 (~400GB/s).  Two 16KB-per-partition loads
                # (halves a=0 / a=1 of the 16-row window; >16KB descriptors
                # also run at half the per-engine rate).
                for a in (0, 1):
                    src = bass.AP(
                        xs,
                        (i0 + a) * GRP,
                        [[GRP, nI], [IMG, CH_PER_CORE], [1, GRP]],
                    )
                    nc.gpsimd.dma_start(out=L[:P, a * GRP : (a + 1) * GRP], in_=src)

                O = op.tile([128, OUT_ROWCH], mybir.dt.float32, name=f"O{i0}", tag="O")
                O_r = O[:P, :].rearrange(
                    "p (j A h B w) -> p A B j h w", j=NB, A=2, h=SH, B=2, w=SW
                )
                L_r = L[:P, :].rearrange("p (A h col) -> p A h col", A=2, h=SH, col=ROW)
                for a in (0, 1):
                    for b in (0, 1):
                        dst_ap = O_r[:, a, b]
                        src_ap = L_r[:, a, :, SW * b : SW * b + NB * SW].rearrange(
                            "p h (j w) -> p j h w", w=SW
                        )
                        nc.vector.tensor_copy(out=dst_ap, in_=src_ap)

                # Four ~16KB-per-partition stores (j-groups), spread across the
                # two HWDGE queues + SWDGE so all rings stay fed.
                # Loads own the gpsimd queue; stores go on sync/scalar only.
                # A store on gpsimd would sit in its FIFO waiting for this
                # chunk's copies, blocking the next chunk's loads behind it.
                engines = [nc.sync, nc.scalar, nc.sync, nc.scalar]
                for gi, (j0, njg) in enumerate(((0, 16), (16, 16), (32, 16), (48, 15))):
                    eng = engines[gi]
                    eng.dma_start(
                        out=bass.AP(
                            out,
                            i0 * OUT_ROWCH + j0 * OUT_BLK,
                            [
                                [OUT_ROWCH, nI],
                                [OUT_IMG, CH_PER_CORE],
                                [1, njg * OUT_BLK],
                            ],
                        ),
                        in_=O[:P, j0 * OUT_BLK : (j0 + njg) * OUT_BLK],
                    )
    nc.compile()
    return nc


def get_nc():
    if "nc" not in _CACHE:
        _CACHE["nc"] = _build_nc()
    return _CACHE["nc"]


def _enable_jax_compile_cache():
    try:
        import jax

        jax.config.update("jax_compilation_cache_dir", "/tmp/jax_neff_cache")
        jax.config.update("jax_persistent_cache_min_entry_size_bytes", -1)
        jax.config.update("jax_persistent_cache_min_compile_time_secs", 0.0)
    except Exception:
        pass


def run_spmd(in_maps, **kwargs):
    from concourse.bass_utils import run_bass_kernel_spmd

    _enable_jax_compile_cache()
    return run_bass_kernel_spmd(
        get_nc(), in_maps, core_ids=list(range(NCORES)), **kwargs
    )


def make_in_maps(x: np.ndarray):
    xs = np.asarray(x, dtype=np.float32).reshape(-1, H, W)
    return [
        {"xs": np.ascontiguousarray(xs[c * CH_PER_CORE : (c + 1) * CH_PER_CORE])}
        for c in range(NCORES)
    ]


def assemble(results, batch_shape):
    outs = np.stack([r["out"] for r in results])  # [8, 4, 3969, 16, 16]
    return outs.reshape(*batch_shape, NB * NB, BH, BW)


def kernel(**inputs) -> np.ndarray:
    x = np.asarray(inputs["x"])
    res = run_spmd(make_in_maps(x))
    return assemble(res.results, x.shape[:2])
